# revision 24
# baseline (speedup 1.0000x reference)
"""Trainium2 Bass kernel for CrossMerge3D.

Input ys: [B=2, S=12, C=96, 32, 32, 32] f32. For each (b, c):
  out = (mA + perm_j(mB) + perm_k(mC)) / 12
where, with the 12 scans split into 3 groups of 4, each group combines as
  m_g = s0 + s1 + flip(s2 + s3)   (flip over the flattened 32^3 volume)
and group B's volume is stored as (j,k,i), group C's as (k,i,j).

Sharding: 8 cores = batch (2) x channel quarters (4) -> 24 channels/core.
No cross-core communication.

Per-core layout: 4 channels x 32 leading-spatial -> 128 SBUF partitions,
1024-wide free dim, 6 macro tiles. Scan-pair 1 MiB loads alternate across
both HWDGE rings at triple buffering (bufs=3) so loads for tile g+3 are
in flight while g is consumed; mid-stream stores go via SWDGE (gpsimd),
keeping the HWDGE rings pure load queues that can never starve behind a
compute dependency. Pair sums on DVE cast to bf16 (tolerance is 2e-2;
bf16 keeps ~4.5e-3). flip = free-dim reversal (pair-sum read APs) +
partition-block reversal, fused with the group combine and the global
1/12 scale as accumulating bf16 matmuls against 1/12-scaled stationaries
(wJ = block anti-diagonal, wI = identity). Groups run C, B, A per tile;
C's (i,j)->(j,i) free permute rides the ACT copy's strided PSUM read,
B's block transpose reads PSUM directly (StreamTranspose f32->f32, no
ACT copy - the scalar engine's only non-DMA op is the early tC copy),
and B's leftover (k,j)->(j,k) permute folds into the final DVE add's
read AP. Drain: the last tile's fA load and fsA pair-sum split into
512-wide halves that pipeline through matmul/final-add/store, and its
two store halves use the (by then idle) HWDGE rings instead of SWDGE.
HW: 117.6 us quiet-host (measured spread 117.6-138.5 across runs; the
shared trn2 host swings all configs, incl. the 134.5 us baseline, by
+/-20 us). Stream bound: 37.75 MB in + 3.15 MB out per core at the
4 KiB-descriptor line rate (~158 ns/desc x 16 SDMA engines ~= 414 GB/s)
= ~98.5 us, plus ~8.5 us fixed preamble (framework barriers + DGE
config) and ~10 us drain (last-load sem latency + fsA/matmul/final-add
chain + store + epilogue barrier).

Known-fragile orderings (measured, do not "fix"): moving either
transpose ahead of rsA in the DVE queue, accumulating mid (tCt) first
in psA, one full-width store per tile, or 512 KiB single-scan loads all
push the Tile scheduler / 8 DMA-sem-lane pipeline into an oscillating
regime 13-23 us slower.
"""

import numpy as np

_B, _S, _C, _D = 2, 12, 96, 32
_NCORE = 8
_CL = _C // 4          # 24 channels per core
_G = _CL // 4          # 6 macro tiles of 4 channels (128 partitions)
_F = _D * _D           # 1024

_nc = None


def _build_program():
    from concourse import bacc, tile, mybir

    f32 = mybir.dt.float32
    bf16 = mybir.dt.bfloat16
    nc = bacc.Bacc(
        "TRN2", target_bir_lowering=False, debug=False, num_devices=_NCORE
    )
    ys = nc.dram_tensor("ys", [_S, _CL, _D, _D, _D], f32, kind="ExternalInput")
    out = nc.dram_tensor("out", [_CL, _D, _D, _D], f32, kind="ExternalOutput")
    ysa = ys.ap()
    outa = out.ap()

    with tile.TileContext(nc) as tc:
        with (
            tc.tile_pool(name="const", bufs=1) as cst,
            tc.tile_pool(name="io", bufs=2) as iop,
            tc.tile_pool(name="tmp", bufs=2) as tmp,
            tc.tile_pool(name="ps", bufs=2, space="PSUM") as ps,
        ):
            scale = 1.0 / 12.0
            # stationaries: 32-block anti-diagonal (x 1/12), identity
            # (x 1/12), and an unscaled identity for the pre-scaled tCt.
            wJ = cst.tile([128, 128], bf16, tag="wJ", name="wJ")
            nc.gpsimd.memset(wJ[:], scale)
            for b in range(4):
                nc.gpsimd.affine_select(
                    out=wJ[32 * b:32 * b + 32, :],
                    in_=wJ[32 * b:32 * b + 32, :],
                    compare_op=mybir.AluOpType.is_equal, fill=0.0,
                    base=-(32 * b + 31), pattern=[[1, 128]],
                    channel_multiplier=1,
                )
            wI = cst.tile([128, 128], bf16, tag="wI", name="wI")
            nc.gpsimd.memset(wI[:], scale)
            nc.gpsimd.affine_select(
                out=wI[:], in_=wI[:],
                compare_op=mybir.AluOpType.is_equal, fill=0.0,
                base=0, pattern=[[1, 128]], channel_multiplier=-1,
            )
            wI1 = cst.tile([128, 128], bf16, tag="wI1", name="wI1")
            nc.gpsimd.memset(wI1[:], 1.0)
            nc.gpsimd.affine_select(
                out=wI1[:], in_=wI1[:],
                compare_op=mybir.AluOpType.is_equal, fill=0.0,
                base=0, pattern=[[1, 128]], channel_multiplier=-1,
            )
            wJ1 = cst.tile([128, 128], bf16, tag="wJ1", name="wJ1")
            nc.gpsimd.memset(wJ1[:], 1.0)
            for b in range(4):
                nc.gpsimd.affine_select(
                    out=wJ1[32 * b:32 * b + 32, :],
                    in_=wJ1[32 * b:32 * b + 32, :],
                    compare_op=mybir.AluOpType.is_equal, fill=0.0,
                    base=-(32 * b + 31), pattern=[[1, 128]],
                    channel_multiplier=1,
                )

            for g in range(_G):
                cs = slice(4 * g, 4 * (g + 1))

                def load_pair(s, tag, eng, split=False):
                    # one 1 MiB DMA per scan-pair. Tile's 8 DMA-sem lanes
                    # throttle issues to ~8 in-flight transfers; 1 MiB
                    # transfers keep that window at ~8 MiB of backlog so
                    # neither ring ever runs dry near the drain (72 x 512
                    # KiB transfers did). The LAST tile's fA (the very
                    # last arrival, feeding the drain chain) goes as two
                    # 512 KiB free-dim halves so the first half's
                    # pair-sum/matmul/final-add starts half a transfer
                    # earlier.
                    t = iop.tile([128, 2 * _F], f32, tag=tag, name=tag,
                                 bufs=3)
                    src = ysa[s:s + 2, cs].rearrange(
                        "s c i j k -> (c i) s (j k)"
                    )
                    dst = t[:].rearrange("p (s f) -> p s f", s=2)
                    if split:
                        for h in (slice(0, _F // 2), slice(_F // 2, _F)):
                            eng.dma_start(out=dst[:, :, h], in_=src[:, :, h])
                    else:
                        eng.dma_start(out=dst, in_=src)
                    return t

                # rev pair first (feeds the longer matmul chain), fwd
                # second; the two pairs of a group go to different rings.
                rC = load_pair(10, "rC", nc.sync)
                fC = load_pair(8, "fC", nc.scalar)
                rB = load_pair(6, "rB", nc.sync)
                fB = load_pair(4, "fB", nc.scalar)
                rA = load_pair(2, "rA", nc.sync)
                fA = load_pair(0, "fA", nc.scalar, split=(g == _G - 1))

                def rev_sum(rt, tag):
                    rs = tmp.tile([128, _F], bf16, tag="rs" + tag,
                                  name="rs" + tag)
                    nc.vector.tensor_add(
                        rs[:], rt[:, 0:_F][:, ::-1], rt[:, _F:2 * _F][:, ::-1])
                    return rs

                def fwd_sum(ft, tag, split=False):
                    fs = tmp.tile([128, _F], bf16, tag="fs" + tag,
                                  name="fs" + tag)
                    if split:
                        # halves pipeline into the matmul/final-add chain
                        # (shortens the end-of-kernel dependent chain).
                        for h in (slice(0, _F // 2), slice(_F // 2, _F)):
                            nc.vector.tensor_add(
                                fs[:, h], ft[:, 0:_F][:, h],
                                ft[:, _F:2 * _F][:, h])
                    else:
                        nc.vector.tensor_add(fs[:], ft[:, 0:_F],
                                             ft[:, _F:2 * _F])
                    return fs

                def pair_sums(rt, ft, tag):
                    return rev_sum(rt, tag), fwd_sum(ft, tag)

                _H = (slice(0, _F // 2), slice(_F // 2, _F))

                def combine(rs, fs, name, wJx, wIx, mid=None):
                    # psum = wJx @ rs + wIx @ fs (+ optional mid member
                    # sharing wIx). Matmuls issue J,J then I,I across the
                    # two PSUM banks to minimize LDWEIGHTS switches.
                    # B and C share a tag so PSUM fits in 8 banks.
                    tag = "psA" if name == "A" else "psBC"
                    pf = ps.tile([128, _F], f32, tag=tag, name=name)
                    for h in _H:
                        nc.tensor.matmul(pf[:, h], wJx[:], rs[:][:, h],
                                         start=True, stop=False)
                    if mid is not None:
                        # both mid members before the fs members: the PE is
                        # in-order, and fs depends on the last-arriving
                        # load, so nothing may queue behind its matmuls.
                        for h in _H:
                            nc.tensor.matmul(pf[:, h], wIx[:], mid[:][:, h],
                                             start=False, stop=False)
                    for h in _H:
                        nc.tensor.matmul(pf[:, h], wIx[:], fs[:][:, h],
                                         start=False, stop=True)
                    return pf

                # C: (i,j)->(j,i) free permute as a strided READ in the
                # PSUM->SBUF copy (contiguous write), then block transpose
                # puts tCt in (j,k) layout. tCt must be bf16 (matmul
                # operand) and StreamTranspose can't cast, so C keeps the
                # ACT copy. C runs UNSCALED (wJ1/wI1); the 1/12 lands
                # when wI accumulates tCt into psA, which lets A's whole
                # group share the wI stationary.
                rsC, fsC = pair_sums(rC, fC, "C")
                psC = combine(rsC, fsC, "C", wJ1, wI1)
                tC = tmp.tile([128, _F], bf16, tag="tC", name="tC")
                nc.scalar.copy(
                    tC[:].rearrange("p (a b) -> p a b", a=_D),
                    psC[:].rearrange("p (a b) -> p a b", a=_D).transpose(
                        [0, 2, 1]),
                )

                # B: j<->i 32x32 block transpose straight out of PSUM,
                # f32 -> f32 (StreamTranspose needs matching dtypes; the
                # final add reads f32 fine). This removes the tB ACT copy
                # so the scalar engine's only compute op per tile is the
                # early-completing tC copy - its HWDGE ring can't starve
                # behind a late dependency. Leftover (k,j) free permute is
                # folded into the final add's read AP.
                rsB, fsB = pair_sums(rB, fB, "B")
                psB = combine(rsB, fsB, "B", wJ, wI)

                # A's rev sum first, then the transposes, then fsA: the
                # DVE wait on the very last load (fA) lands AFTER the
                # transposes in queue order, and the transposes fill the
                # DVE gap between rA's and fA's arrivals. (Moving either
                # transpose ahead of rsA makes the Tile scheduler emit a
                # pathological DVE order - measured 19-23us slower.)
                rsA = rev_sum(rA, "A")
                tCt = tmp.tile([128, _F], bf16, tag="tCt", name="tCt")
                nc.vector.transpose(tCt[:], tC[:])
                tBt = tmp.tile([128, _F], f32, tag="tBt", name="tBt")
                nc.vector.transpose(tBt[:], psB[:])
                fsA = fwd_sum(fA, "A", split=True)

                # A accumulates C's contribution (unscaled tCt picks up
                # its 1/12 from wI here) plus its own scans; A's fwd pair
                # is the last load, feeding only the shallow wI@fsA ->
                # final add path (no LDWEIGHTS switch left in the drain).
                psA = combine(rsA, fsA, "A", wJ, wI, mid=tCt)

                # out = psA + tBt read as (j,k); f32 write, no scale op.
                # Done in 512-wide halves, each stored immediately, to
                # shorten the end-of-kernel drain. Mid-stream BOTH halves
                # store via SWDGE - the sync/scalar queues then carry
                # nothing but loads, so a late final add can never block
                # the next tile's load issue. The LAST tile uses both
                # HWDGE rings instead (no load left to displace, and
                # HWDGE's ~0.6us first-byte latency beats SWDGE's Q7
                # emission in the drain).
                o = tmp.tile([128, _F], f32, tag="o", name="o")
                ov = o[:].rearrange("p (j k) -> p j k", j=_D)
                pv = psA[:].rearrange("p (j k) -> p j k", j=_D)
                bv = tBt[:].rearrange("p (k j) -> p j k", k=_D)
                od = outa[cs].rearrange("c i j k -> (c i) (j k)")
                last = g == _G - 1
                for jh, eng in (
                    (slice(0, _D // 2), nc.sync if last else nc.gpsimd),
                    (slice(_D // 2, _D), nc.scalar if last else nc.gpsimd),
                ):
                    nc.vector.tensor_add(ov[:, jh], pv[:, jh], bv[:, jh])
                    eng.dma_start(
                        out=od[:, _F // 2 * (jh.start != 0):][:, :_F // 2],
                        in_=o[:, _F // 2 * (jh.start != 0):][:, :_F // 2],
                    )

    nc.compile()
    return nc


def kernel(ys):
    global _nc
    ys = np.ascontiguousarray(ys, dtype=np.float32)
    assert ys.shape == (_B, _S, _C, _D, _D, _D), ys.shape

    if _nc is None:
        _nc = _build_program()

    from concourse.bass_utils import run_bass_kernel_spmd

    in_maps = []
    for r in range(_NCORE):
        b, q = divmod(r, 4)
        shard = np.ascontiguousarray(ys[b, :, q * _CL:(q + 1) * _CL])
        in_maps.append({"ys": shard})

    res = run_bass_kernel_spmd(_nc, in_maps, list(range(_NCORE)))

    out = np.empty((_B, _C, _D, _D, _D), np.float32)
    for r in range(_NCORE):
        b, q = divmod(r, 4)
        out[b, q * _CL:(q + 1) * _CL] = np.asarray(
            res.results[r]["out"]).astype(np.float32)

    if res.exec_time_ns is not None:
        print(f"HW exec time: {res.exec_time_ns} ns")
    return out



# revision 25
# speedup vs baseline: 1.0030x; 1.0030x over previous
"""Trainium2 Bass kernel for CrossMerge3D.

Input ys: [B=2, S=12, C=96, 32, 32, 32] f32. For each (b, c):
  out = (mA + perm_j(mB) + perm_k(mC)) / 12
where, with the 12 scans split into 3 groups of 4, each group combines as
  m_g = s0 + s1 + flip(s2 + s3)   (flip over the flattened 32^3 volume)
and group B's volume is stored as (j,k,i), group C's as (k,i,j).

Sharding: 8 cores = batch (2) x channel quarters (4) -> 24 channels/core.
No cross-core communication.

Per-core layout: 4 channels x 32 leading-spatial -> 128 SBUF partitions,
1024-wide free dim, 6 macro tiles. Scan-pair 1 MiB loads alternate across
both HWDGE rings at triple buffering (bufs=3) so loads for tile g+3 are
in flight while g is consumed; mid-stream stores go via SWDGE (gpsimd),
keeping the HWDGE rings pure load queues that can never starve behind a
compute dependency. Pair sums on DVE cast to bf16 (tolerance is 2e-2;
bf16 keeps ~4.5e-3). flip = free-dim reversal (pair-sum read APs) +
partition-block reversal, fused with the group combine and the global
1/12 scale as accumulating bf16 matmuls against 1/12-scaled stationaries
(wJ = block anti-diagonal, wI = identity). Groups run C, B, A per tile;
C's (i,j)->(j,i) free permute rides the ACT copy's strided PSUM read,
B's block transpose reads PSUM directly (StreamTranspose f32->f32, no
ACT copy - the scalar engine's only non-DMA op is the early tC copy),
and B's leftover (k,j)->(j,k) permute folds into the final DVE add's
read AP. Drain: the last tile's fA load and fsA pair-sum split into
512-wide halves that pipeline through matmul/final-add/store, and its
two store halves use the (by then idle) HWDGE rings instead of SWDGE.
HW: 117.6 us quiet-host (measured spread 117.6-138.5 across runs; the
shared trn2 host swings all configs, incl. the 134.5 us baseline, by
+/-20 us). Stream bound: 37.75 MB in + 3.15 MB out per core at the
4 KiB-descriptor line rate (~158 ns/desc x 16 SDMA engines ~= 414 GB/s)
= ~98.5 us, plus ~8.5 us fixed preamble (framework barriers + DGE
config) and ~10 us drain (last-load sem latency + fsA/matmul/final-add
chain + store + epilogue barrier).

Known-fragile orderings (measured, do not "fix"): moving either
transpose ahead of rsA in the DVE queue, accumulating mid (tCt) first
in psA, one full-width store per tile, or 512 KiB single-scan loads all
push the Tile scheduler / 8 DMA-sem-lane pipeline into an oscillating
regime 13-23 us slower.
"""

import numpy as np

_B, _S, _C, _D = 2, 12, 96, 32
_NCORE = 8
_CL = _C // 4          # 24 channels per core
_G = _CL // 4          # 6 macro tiles of 4 channels (128 partitions)
_F = _D * _D           # 1024

_nc = None


def _build_program():
    from concourse import bacc, tile, mybir

    f32 = mybir.dt.float32
    bf16 = mybir.dt.bfloat16
    nc = bacc.Bacc(
        "TRN2", target_bir_lowering=False, debug=False, num_devices=_NCORE
    )
    ys = nc.dram_tensor("ys", [_S, _CL, _D, _D, _D], f32, kind="ExternalInput")
    out = nc.dram_tensor("out", [_CL, _D, _D, _D], f32, kind="ExternalOutput")
    ysa = ys.ap()
    outa = out.ap()

    with tile.TileContext(nc) as tc:
        with (
            tc.tile_pool(name="const", bufs=1) as cst,
            tc.tile_pool(name="io", bufs=2) as iop,
            tc.tile_pool(name="tmp", bufs=2) as tmp,
            tc.tile_pool(name="ps", bufs=2, space="PSUM") as ps,
        ):
            scale = 1.0 / 12.0
            # stationaries: 32-block anti-diagonal (x 1/12), identity
            # (x 1/12), and an unscaled identity for the pre-scaled tCt.
            wJ = cst.tile([128, 128], bf16, tag="wJ", name="wJ")
            nc.gpsimd.memset(wJ[:], scale)
            for b in range(4):
                nc.gpsimd.affine_select(
                    out=wJ[32 * b:32 * b + 32, :],
                    in_=wJ[32 * b:32 * b + 32, :],
                    compare_op=mybir.AluOpType.is_equal, fill=0.0,
                    base=-(32 * b + 31), pattern=[[1, 128]],
                    channel_multiplier=1,
                )
            wI = cst.tile([128, 128], bf16, tag="wI", name="wI")
            nc.gpsimd.memset(wI[:], scale)
            nc.gpsimd.affine_select(
                out=wI[:], in_=wI[:],
                compare_op=mybir.AluOpType.is_equal, fill=0.0,
                base=0, pattern=[[1, 128]], channel_multiplier=-1,
            )
            wI1 = cst.tile([128, 128], bf16, tag="wI1", name="wI1")
            nc.gpsimd.memset(wI1[:], 1.0)
            nc.gpsimd.affine_select(
                out=wI1[:], in_=wI1[:],
                compare_op=mybir.AluOpType.is_equal, fill=0.0,
                base=0, pattern=[[1, 128]], channel_multiplier=-1,
            )
            wJ1 = cst.tile([128, 128], bf16, tag="wJ1", name="wJ1")
            nc.gpsimd.memset(wJ1[:], 1.0)
            for b in range(4):
                nc.gpsimd.affine_select(
                    out=wJ1[32 * b:32 * b + 32, :],
                    in_=wJ1[32 * b:32 * b + 32, :],
                    compare_op=mybir.AluOpType.is_equal, fill=0.0,
                    base=-(32 * b + 31), pattern=[[1, 128]],
                    channel_multiplier=1,
                )

            for g in range(_G):
                cs = slice(4 * g, 4 * (g + 1))

                def load_pair(s, tag, eng, split=False):
                    # one 1 MiB DMA per scan-pair. Tile's 8 DMA-sem lanes
                    # throttle issues to ~8 in-flight transfers; 1 MiB
                    # transfers keep that window at ~8 MiB of backlog so
                    # neither ring ever runs dry near the drain (72 x 512
                    # KiB transfers did). The LAST tile's fA (the very
                    # last arrival, feeding the drain chain) goes as two
                    # 512 KiB free-dim halves so the first half's
                    # pair-sum/matmul/final-add starts half a transfer
                    # earlier.
                    t = iop.tile([128, 2 * _F], f32, tag=tag, name=tag,
                                 bufs=3)
                    src = ysa[s:s + 2, cs].rearrange(
                        "s c i j k -> (c i) s (j k)"
                    )
                    dst = t[:].rearrange("p (s f) -> p s f", s=2)
                    if split:
                        for h in (slice(0, _F // 2), slice(_F // 2, _F)):
                            eng.dma_start(out=dst[:, :, h], in_=src[:, :, h])
                    else:
                        eng.dma_start(out=dst, in_=src)
                    return t

                # rev pair first (feeds the longer matmul chain), fwd
                # second; the two pairs of a group go to different rings.
                rC = load_pair(10, "rC", nc.sync)
                fC = load_pair(8, "fC", nc.scalar)
                rB = load_pair(6, "rB", nc.sync)
                fB = load_pair(4, "fB", nc.scalar)
                rA = load_pair(2, "rA", nc.sync)
                fA = load_pair(0, "fA", nc.scalar, split=(g == _G - 1))

                def rev_sum(rt, tag):
                    rs = tmp.tile([128, _F], bf16, tag="rs" + tag,
                                  name="rs" + tag)
                    nc.vector.tensor_add(
                        rs[:], rt[:, 0:_F][:, ::-1], rt[:, _F:2 * _F][:, ::-1])
                    return rs

                def fwd_sum(ft, tag, split=False):
                    fs = tmp.tile([128, _F], bf16, tag="fs" + tag,
                                  name="fs" + tag)
                    if split:
                        # halves pipeline into the matmul/final-add chain
                        # (shortens the end-of-kernel dependent chain).
                        for h in (slice(0, _F // 2), slice(_F // 2, _F)):
                            nc.vector.tensor_add(
                                fs[:, h], ft[:, 0:_F][:, h],
                                ft[:, _F:2 * _F][:, h])
                    else:
                        nc.vector.tensor_add(fs[:], ft[:, 0:_F],
                                             ft[:, _F:2 * _F])
                    return fs

                def pair_sums(rt, ft, tag):
                    return rev_sum(rt, tag), fwd_sum(ft, tag)

                _H = (slice(0, _F // 2), slice(_F // 2, _F))

                def combine(rs, fs, name, wJx, wIx, mid=None):
                    # psum = wJx @ rs + wIx @ fs (+ optional mid member
                    # sharing wIx). Matmuls issue J,J then I,I across the
                    # two PSUM banks to minimize LDWEIGHTS switches.
                    # B and C share a tag so PSUM fits in 8 banks.
                    tag = "psA" if name == "A" else "psBC"
                    pf = ps.tile([128, _F], f32, tag=tag, name=name)
                    for h in _H:
                        nc.tensor.matmul(pf[:, h], wJx[:], rs[:][:, h],
                                         start=True, stop=False)
                    if mid is not None:
                        # both mid members before the fs members: the PE is
                        # in-order, and fs depends on the last-arriving
                        # load, so nothing may queue behind its matmuls.
                        for h in _H:
                            nc.tensor.matmul(pf[:, h], wIx[:], mid[:][:, h],
                                             start=False, stop=False)
                    for h in _H:
                        nc.tensor.matmul(pf[:, h], wIx[:], fs[:][:, h],
                                         start=False, stop=True)
                    return pf

                # C: (i,j)->(j,i) free permute as a strided READ in the
                # PSUM->SBUF copy (contiguous write), then block transpose
                # puts tCt in (j,k) layout. tCt must be bf16 (matmul
                # operand) and StreamTranspose can't cast, so C keeps the
                # ACT copy. C runs UNSCALED (wJ1/wI1); the 1/12 lands
                # when wI accumulates tCt into psA, which lets A's whole
                # group share the wI stationary.
                rsC, fsC = pair_sums(rC, fC, "C")
                psC = combine(rsC, fsC, "C", wJ1, wI1)
                tC = tmp.tile([128, _F], bf16, tag="tC", name="tC")
                nc.scalar.copy(
                    tC[:].rearrange("p (a b) -> p a b", a=_D),
                    psC[:].rearrange("p (a b) -> p a b", a=_D).transpose(
                        [0, 2, 1]),
                )

                # B: j<->i 32x32 block transpose straight out of PSUM,
                # f32 -> f32 (StreamTranspose needs matching dtypes; the
                # final add reads f32 fine). This removes the tB ACT copy
                # so the scalar engine's only compute op per tile is the
                # early-completing tC copy - its HWDGE ring can't starve
                # behind a late dependency. Leftover (k,j) free permute is
                # folded into the final add's read AP.
                rsB, fsB = pair_sums(rB, fB, "B")
                psB = combine(rsB, fsB, "B", wJ, wI)

                # A's rev sum first, then the transposes, then fsA: the
                # DVE wait on the very last load (fA) lands AFTER the
                # transposes in queue order, and the transposes fill the
                # DVE gap between rA's and fA's arrivals. (Moving either
                # transpose ahead of rsA makes the Tile scheduler emit a
                # pathological DVE order - measured 19-23us slower.)
                rsA = rev_sum(rA, "A")
                tCt = tmp.tile([128, _F], bf16, tag="tCt", name="tCt")
                nc.vector.transpose(tCt[:], tC[:])
                tBt = tmp.tile([128, _F], f32, tag="tBt", name="tBt")
                nc.vector.transpose(tBt[:], psB[:])
                fsA = fwd_sum(fA, "A", split=True)

                # A accumulates C's contribution (unscaled tCt picks up
                # its 1/12 from wI here) plus its own scans; A's fwd pair
                # is the last load, feeding only the shallow wI@fsA ->
                # final add path (no LDWEIGHTS switch left in the drain).
                psA = combine(rsA, fsA, "A", wJ, wI, mid=tCt)

                # out = psA + tBt read as (j,k); f32 write, no scale op.
                # Done in 512-wide halves, each stored immediately, to
                # shorten the end-of-kernel drain. Mid-stream BOTH halves
                # store via SWDGE - the sync/scalar queues then carry
                # nothing but loads, so a late final add can never block
                # the next tile's load issue. The LAST tile uses both
                # HWDGE rings instead (no load left to displace, and
                # HWDGE's ~0.6us first-byte latency beats SWDGE's Q7
                # emission in the drain).
                o = tmp.tile([128, _F], f32, tag="o", name="o")
                ov = o[:].rearrange("p (j k) -> p j k", j=_D)
                pv = psA[:].rearrange("p (j k) -> p j k", j=_D)
                bv = tBt[:].rearrange("p (k j) -> p j k", k=_D)
                od = outa[cs].rearrange("c i j k -> (c i) (j k)")
                if g == _G - 1:
                    # drain: final adds + stores in QUARTERS on the (by
                    # now idle) HWDGE rings - earlier quarters' store
                    # data overlaps the remaining adds and the LAST
                    # store is only 128 KiB, so its data + write receipt
                    # (what the end barrier waits on) lands ~1.3us
                    # earlier than a 256 KiB half would.
                    for q, eng in enumerate(
                            (nc.sync, nc.scalar, nc.sync, nc.scalar)):
                        jq = slice(_D // 4 * q, _D // 4 * (q + 1))
                        nc.vector.tensor_add(ov[:, jq], pv[:, jq],
                                             bv[:, jq])
                        eng.dma_start(
                            out=od[:, _F // 4 * q:][:, :_F // 4],
                            in_=o[:, _F // 4 * q:][:, :_F // 4],
                        )
                else:
                    for jh in (slice(0, _D // 2), slice(_D // 2, _D)):
                        nc.vector.tensor_add(ov[:, jh], pv[:, jh],
                                             bv[:, jh])
                        nc.gpsimd.dma_start(
                            out=od[:, _F // 2 * (jh.start != 0):][:, :_F // 2],
                            in_=o[:, _F // 2 * (jh.start != 0):][:, :_F // 2],
                        )

    nc.compile()
    return nc


def kernel(ys):
    global _nc
    ys = np.ascontiguousarray(ys, dtype=np.float32)
    assert ys.shape == (_B, _S, _C, _D, _D, _D), ys.shape

    if _nc is None:
        _nc = _build_program()

    from concourse.bass_utils import run_bass_kernel_spmd

    in_maps = []
    for r in range(_NCORE):
        b, q = divmod(r, 4)
        shard = np.ascontiguousarray(ys[b, :, q * _CL:(q + 1) * _CL])
        in_maps.append({"ys": shard})

    res = run_bass_kernel_spmd(_nc, in_maps, list(range(_NCORE)))

    out = np.empty((_B, _C, _D, _D, _D), np.float32)
    for r in range(_NCORE):
        b, q = divmod(r, 4)
        out[b, q * _CL:(q + 1) * _CL] = np.asarray(
            res.results[r]["out"]).astype(np.float32)

    if res.exec_time_ns is not None:
        print(f"HW exec time: {res.exec_time_ns} ns")
    return out

